# revision 29
# baseline (speedup 1.0000x reference)
"""Single-head causal attention on 8 Trainium2 NeuronCores.

Problem: x[8, 4096, 384], Wq/Wk/Wv[384, 64] ->
    out[b] = softmax(causal((x[b]Wq)(x[b]Wk)^T / sqrt(384))) @ (x[b]Wv)

Sharding: data-parallel over batch - core i computes batch element i.
Weights are replicated to every core.

Host-side marshaling (layout only, no FLOPs): x is pre-transposed and
cast to fp16 (x_t[b] = x[b].T), Wq|Wk are packed into one [384,128]
weight, and a 128x128 causal triangle mask is passed as a const.  All
projections, scores, softmax and the PV contraction run on device.

Per-core kernel layout (all matmuls contract over the partition axis):
  - X^T tiles [c=128, t] are DMA'd directly from the host-transposed
    x_t (no PE transposes).
  - One fused matmul per embed chunk produces [Q^T; K^T] stacked on
    partitions (wqk = [Wq | Wk]): psqk[0:64] = Q^T, psqk[64:128] = K^T.
    Q^T is duplicated into both partition halves of qt2, K^T chunks go
    to alternating halves of kt2, so score matmuls run row-packed
    two-at-a-time in the 128x128 PE array (they execute concurrently).
  - V^T [64, t] from a matmul; transposed to natural V via the DMA xbar
    (dma_start_transpose) into vext[t=128, 65] = [V | 1] (ones column ->
    softmax denominator).
  - Scores are computed TRANSPOSED: S^T[s, q] = K Q^T so the softmax
    sum over s becomes a matmul partition axis and P^T feeds the PV
    matmul directly:  O^T[h+1, q] += V_ext[s,:]^T @ P^T[s, q].
  - Causal handling: diagonal s-chunks only stream columns >= 128*d
    (nothing is computed left of the triangle), and only the 128x128
    triangle block is masked (one fp16 multiply with a const mask).
  - exp via ScalarE activation over 3-chunk groups (PSUM-bank limited);
    no max subtraction (|scores/sqrt(C)| is small, fp32 exp is safe).
  - PV matmuls are emitted one group late (software pipelining) so the
    PE never idles waiting for the exp of the group it just computed.
  - O^T is transposed back to [q=128, 65] via the DMA xbar; column 64
    holds the row sums; divide and DMA out.
"""

import sys

if "/opt/trn_rl_repo" not in sys.path:
    sys.path.insert(0, "/opt/trn_rl_repo")

import numpy as np

import concourse.bass as bass  # noqa: F401  (AP types used implicitly)
import concourse.tile as tile
from concourse import bacc, mybir
from concourse.bass import ds
from concourse.bass_utils import run_bass_kernel_spmd
from concourse.masks import make_identity

B = 8
T_FULL = 4096
C = 384
H = 64
P = 128
TQ = 512  # q-block width
GRP = 3  # score chunks per exp activation (PSUM: 3 banks per pss buffer)
SCALE = 1.0 / float(np.sqrt(C))
F32 = mybir.dt.float32
F16 = mybir.dt.float16
EXP = mybir.ActivationFunctionType.Exp


def build_nc(T=T_FULL):
    """Build the per-core Bass program (same program on all 8 cores)."""
    NT = T // P  # number of 128-row s-chunks
    NQ = T // TQ  # number of 512-col q-blocks
    CC = C // P  # 3 embed chunks
    SUB = TQ // P  # 4 sub-tiles per block

    nc = bacc.Bacc(
        "TRN2",
        target_bir_lowering=False,
        debug=False,
        enable_asserts=True,
        num_devices=B,
    )
    xt_ap = nc.dram_tensor("x_t", [C, T], F16, kind="ExternalInput").ap()
    wqk_ap = nc.dram_tensor("wqk", [C, P], F16, kind="ExternalInput").ap()
    wv_ap = nc.dram_tensor("wv", [C, H], F16, kind="ExternalInput").ap()
    tri_ap = nc.dram_tensor("tri", [P, P], F16, kind="ExternalInput").ap()
    out_ap = nc.dram_tensor("out", [T, H], F32, kind="ExternalOutput").ap()

    x_re = xt_ap.rearrange("(c p) t -> p c t", p=P)  # [128, 3, T]
    out_re = out_ap.rearrange("(n p) h -> p n h", p=P)  # [128, NT, 64]

    with tile.TileContext(nc) as tc:
        with (
            tc.tile_pool(name="consts", bufs=1) as consts,
            tc.tile_pool(name="xtp", bufs=4) as xtp,
            tc.tile_pool(name="qkt", bufs=1) as qktp,
            tc.tile_pool(name="vextp", bufs=1) as vextp,
            tc.tile_pool(name="vtp", bufs=2) as vtp,
            tc.tile_pool(name="ptp", bufs=3) as ptp,
            tc.tile_pool(name="otp", bufs=2) as otp,
            tc.tile_pool(name="op", bufs=2) as op_,
            tc.tile_pool(name="rvp", bufs=2) as rvp,
            tc.tile_pool(name="pswide", bufs=2, space="PSUM") as pswide,
            tc.tile_pool(name="psop", bufs=1, space="PSUM") as psop,
            tc.tile_pool(name="psqkv", bufs=1, space="PSUM") as psqkv,
        ):
            wqk_sb = consts.tile([P, CC, P], F16)
            nc.sync.dma_start(
                out=wqk_sb, in_=wqk_ap.rearrange("(c p) m -> p c m", p=P)
            )
            wv_sb = consts.tile([P, CC, H], F16)
            nc.scalar.dma_start(out=wv_sb, in_=wv_ap.rearrange("(c p) h -> p c h", p=P))
            tri_sb = consts.tile([P, P], F16)
            nc.scalar.dma_start(out=tri_sb, in_=tri_ap)
            ident_h = consts.tile([P, P], F16)
            make_identity(nc, ident_h)

            # vext[:, c, 0:64] = V chunk c (written by DMA transpose);
            # col 64 = ones (memset now, never overwritten).
            vext = vextp.tile([P, NT, 72], F16)
            nc.vector.memset(vext, 1.0)

            # qt2: Q^T duplicated in both partition halves.
            # kt2: K^T chunk c lives at partitions 64*(c%2), col (c//2)*128.
            qt2 = qktp.tile([P, T], F16, tag="qt")
            kt2 = qktp.tile([P, (NT // 2) * P], F16, tag="kt")

            def phase1_gen(j):
                """Load X^T cols [512j, 512j+512), produce Q^T, K^T, V.

                Yields between chunks so the driver can spread this work
                into the gaps of the attention loop.
                """
                xt = xtp.tile([P, CC, TQ], F16, tag="xt", name=f"xt{j}")
                nc.gpsimd.dma_start(out=xt, in_=x_re[:, :, TQ * j : TQ * (j + 1)])
                yield
                blk = ds(j * TQ, TQ)
                psqk = psqkv.tile([P, TQ], F32, tag="qkv", name=f"psqk{j}")
                for c in range(CC):
                    nc.tensor.matmul(
                        psqk,
                        lhsT=wqk_sb[:, c, :],
                        rhs=xt[:, c, :],
                        start=(c == 0),
                        stop=(c == CC - 1),
                    )
                yield
                # One fast staging cast releases the qkv PSUM bank quickly;
                # the qt2/kt2 layout copies then run from SBUF at 4x.
                qkst = xtp.tile([P, TQ], F16, tag="qkst", name=f"qkst{j}")
                nc.vector.tensor_copy(out=qkst, in_=psqk)
                yield
                nc.vector.tensor_copy(out=qt2[0:H, blk], in_=qkst[0:H, :])
                nc.vector.tensor_copy(out=qt2[H:P, blk], in_=qkst[0:H, :])
                yield
                for st in range(SUB):
                    c = SUB * j + st
                    half = H * (c % 2)
                    nc.vector.tensor_copy(
                        out=kt2[half : half + H, (c // 2) * P : (c // 2 + 1) * P],
                        in_=qkst[H:P, st * P : (st + 1) * P],
                    )
                yield
                psv = psqkv.tile([H, TQ], F32, tag="qkv", name=f"psv{j}")
                for c in range(CC):
                    nc.tensor.matmul(
                        psv,
                        lhsT=wv_sb[:, c, :],
                        rhs=xt[:, c, :],
                        start=(c == 0),
                        stop=(c == CC - 1),
                    )
                yield
                vt = vtp.tile([H, TQ], F16, tag="vt", name=f"vt{j}")
                nc.vector.tensor_copy(out=vt, in_=psv)
                # The HW xbar ignores strides between out chunks (writes
                # compactly), so transpose into a contiguous staging tile
                # and fan out to vext's strided layout with one DVE copy.
                vst = vtp.tile([P, SUB, H], F16, tag="vst", name=f"vst{j}")
                nc.scalar.dma_start_transpose(out=vst, in_=vt)
                nc.vector.tensor_copy(
                    out=vext[:, SUB * j : SUB * (j + 1), 0:H], in_=vst
                )
                yield

            N1_CHUNKS = 7

            def phase2(j, pump):
                """Attention for q cols [512j, 512j+512).  pump(done, total)
                advances the interleaved next-block phase-1 generator."""
                nchunks = SUB * (j + 1)
                groups = [
                    list(range(g, min(g + GRP, nchunks)))
                    for g in range(0, nchunks, GRP)
                ]
                pso = psop.tile([H + 1, TQ], F32, tag="pso", name=f"pso{j}")
                pending = None
                nsteps = len(groups) + 1
                for gi, grp in enumerate(groups):
                    pss = pswide.tile([P, GRP * TQ], F32, tag="wide", name=f"pss{j}_{gi}")
                    pt = ptp.tile([P, GRP * TQ], F16, tag="pt", name=f"pt{j}_{gi}")
                    for k, c in enumerate(grp):
                        half = H * (c % 2)
                        nc.tensor.matmul(
                            pss[:, TQ * k : TQ * (k + 1)],
                            lhsT=kt2[half : half + H, (c // 2) * P : (c // 2 + 1) * P],
                            rhs=qt2[half : half + H, TQ * j : TQ * (j + 1)],
                            start=True,
                            stop=True,
                            tile_position=(half, 0),
                        )
                    end = TQ * len(grp)
                    nc.scalar.activation(
                        out=pt[:, 0:end],
                        in_=pss[:, 0:end],
                        func=EXP,
                        scale=SCALE,
                    )
                    for k, c in enumerate(grp):
                        d = c - SUB * j
                        if d >= 0:
                            sl = ds(TQ * k + P * d, P)
                            nc.vector.tensor_mul(
                                out=pt[:, sl], in0=pt[:, sl], in1=tri_sb
                            )
                    if pending is not None:
                        for k, c, ppt in pending:
                            d = c - SUB * j
                            s0 = P * d if d >= 0 else 0
                            nc.tensor.matmul(
                                pso[:, s0:TQ],
                                lhsT=vext[:, c, 0 : H + 1],
                                rhs=ppt[:, TQ * k + s0 : TQ * (k + 1)],
                                start=(c == 0),
                                stop=(c == nchunks - 1),
                            )
                    pending = [(k, c, pt) for k, c in enumerate(grp)]
                    pump(gi + 1, nsteps)
                for k, c, ppt in pending:
                    d = c - SUB * j
                    s0 = P * d if d >= 0 else 0
                    nc.tensor.matmul(
                        pso[:, s0:TQ],
                        lhsT=vext[:, c, 0 : H + 1],
                        rhs=ppt[:, TQ * k + s0 : TQ * (k + 1)],
                        start=(c == 0),
                        stop=(c == nchunks - 1),
                    )
                pump(nsteps, nsteps)
                # O^T -> O; col 64 = row sums.  Blocks 0..NQ-2 use the DMA
                # xbar (cheap, overlapped); the last block uses PE transposes
                # to avoid the ~2.3us DMA latency on the serial tail.
                ot = otp.tile([80, TQ], F16, tag="ot", name=f"ot{j}")
                if j < NQ - 1:
                    nc.vector.memset(ot[H:80, :], 0.0)
                nc.vector.tensor_copy(out=ot[0 : H + 1, :], in_=pso)
                if j < NQ - 1:
                    o = op_.tile([P, SUB, 80], F16, tag="o", name=f"o{j}")
                    nc.sync.dma_start_transpose(out=o, in_=ot)
                else:
                    pstr = psqkv.tile([P, SUB, H + 2], F16, tag="qkv", name="pstr")
                    for i in range(SUB):
                        nc.tensor.transpose(
                            pstr[:, i, 0 : H + 1],
                            ot[0 : H + 1, i * P : (i + 1) * P],
                            ident_h[0 : H + 1, 0 : H + 1],
                        )
                    o = op_.tile([P, SUB, 80], F16, tag="o", name=f"o{j}")
                    nc.vector.tensor_copy(
                        out=o[:, :, 0 : H + 1], in_=pstr[:, :, 0 : H + 1]
                    )
                rv = rvp.tile([P, SUB], F32, tag="rv", name=f"rv{j}")
                nc.vector.reciprocal(out=rv, in_=o[:, :, H : H + 1])
                o32 = op_.tile([P, SUB, H], F32, tag="o32", name=f"o32{j}")
                for i in range(SUB):
                    nc.vector.tensor_scalar_mul(
                        out=o32[:, i, :],
                        in0=o[:, i, 0:H],
                        scalar1=rv[:, i : i + 1],
                    )
                nc.sync.dma_start(
                    out=out_re[:, SUB * j : SUB * (j + 1), :], in_=o32
                )

            LEAD = 3
            gens = [phase1_gen(i) for i in range(LEAD)]
            live = list(gens)
            while live:
                for g in list(live):
                    try:
                        next(g)
                    except StopIteration:
                        live.remove(g)
            for j in range(NQ):
                gen = phase1_gen(j + LEAD) if j + LEAD < NQ else None
                adv = {"n": 0}

                def pump(done, total, gen=gen, adv=adv):
                    if gen is None:
                        return
                    want = done * N1_CHUNKS // total
                    while adv["n"] < want:
                        try:
                            next(gen)
                        except StopIteration:
                            break
                        adv["n"] += 1

                phase2(j, pump)
                if gen is not None:
                    for _ in gen:
                        pass

    nc.compile()
    return nc


_NC_CACHE = {}


def _get_nc():
    if "nc" not in _NC_CACHE:
        _NC_CACHE["nc"] = build_nc()
    return _NC_CACHE["nc"]


def kernel(x, Wk, Wq, Wv, _trace=False, _trace_kwargs=None):
    x = np.asarray(x, dtype=np.float32)
    Wk = np.asarray(Wk, dtype=np.float32)
    Wq = np.asarray(Wq, dtype=np.float32)
    Wv = np.asarray(Wv, dtype=np.float32)
    # Host-side layout marshaling (no FLOPs): transpose x per batch,
    # cast to fp16, pack Wq|Wk, build the causal triangle mask.
    x_t = np.ascontiguousarray(x.transpose(0, 2, 1)).astype(np.float16)
    wqk = np.ascontiguousarray(
        np.concatenate([Wq, Wk], axis=1), dtype=np.float16
    )
    wv = np.ascontiguousarray(Wv, dtype=np.float16)
    tri = np.triu(np.ones((P, P), dtype=np.float16))  # tri[s, u] = u >= s
    nc = _get_nc()
    in_maps = [
        {"x_t": x_t[b], "wqk": wqk, "wv": wv, "tri": tri} for b in range(B)
    ]
    res = run_bass_kernel_spmd(
        nc, in_maps, list(range(B)), trace=_trace, **(_trace_kwargs or {})
    )
    out = np.stack([res.results[b]["out"] for b in range(B)], axis=0)
    if _trace:
        return out, res
    return out
